# revision 47
# baseline (speedup 1.0000x reference)
"""Trainium2 Bass kernel for nn_LAMME (conv3x3 + LAM temporal attention + ME gate).

Data-parallel over 8 NeuronCores: each core processes one clip of t=8 frames
(c=256, h=w=56). Single fused kernel per core.

Structure (v4):
 - prepass: pooled conv statistics computed EXACTLY from the input via
   edge-corrected strip sums (sum_hw(conv(x)) = sum_taps w.T_strip), so all
   LAM/ME gates are ready ~150us in and the gating phase overlaps the conv.
 - frames 0-3: direct conv (18 accumulated matmuls / tile, ACT PSUM->SBUF
   copy); the small-op gate chain is interleaved into frame 2's tile slots,
   gated phase-2 of frames 0-2 into frame 3's slots.
 - frames 4-7: 1D Winograd F(2,3) along H in a (30 MM + 2 DVE op) form:
   E=M0+M1 and O=M1-M3 accumulate directly in PSUM with host-folded weights,
   M2 materialized once; y_even=E+M2, y_odd=O-M2 are single tensor_tensor
   ops.  Cuts PE work per frame ~17% at a DVE cost that fits under the PE.
 - phase 2 (fin = g0*o[f-1]+g1*o[f]+g2*o[f+1]+goffs) is all-DVE
   (tensor_scalar + 2 in-place scalar_tensor_tensor), chunk-interleaved
   behind the conv tiles it depends on; output DMA split across 2 queues.
"""
import sys
for p in ('/opt/trn_rl_repo',):
    if p not in sys.path:
        sys.path.insert(0, p)

import numpy as np
import ml_dtypes

import concourse.bacc as bacc
import concourse.mybir as mybir
import concourse.tile as tile
from concourse.bass_utils import run_bass_kernel_spmd

F32 = mybir.dt.float32
BF16 = mybir.dt.bfloat16
AF = mybir.ActivationFunctionType
OP = mybir.AluOpType

T = 8          # frames per clip (= clips per core after sharding)
NCORES = 8
NSP = 7        # spatial tiles per frame (56 rows / 8)
TW = 448       # 8 rows x 56 cols per tile
HP = 58        # padded spatial width
PADSZ = HP * HP  # 3364
WINO_FRAMES = (4, 5, 6, 7)

_CACHE = {}


def _build(me):
    nc = bacc.Bacc("TRN2", target_bir_lowering=False, debug=False)

    x_d = nc.dram_tensor("x", [T, 2, 128, PADSZ], BF16, kind="ExternalInput")
    wt_d = nc.dram_tensor("wt", [128, 36, 128], BF16, kind="ExternalInput")
    wg_d = nc.dram_tensor("wg", [128, 48, 128], BF16, kind="ExternalInput")
    lamw_d = nc.dram_tensor("lamw", [4, 128, 128], BF16, kind="ExternalInput")
    w1t_d = nc.dram_tensor("w1t", [8, 16], BF16, kind="ExternalInput")
    w2t_d = nc.dram_tensor("w2t", [16, 3], BF16, kind="ExternalInput")
    bns_d = nc.dram_tensor("bns", [16, 1], F32, kind="ExternalInput")
    bnsh_d = nc.dram_tensor("bnsh", [16, 1], F32, kind="ExternalInput")
    netb_d = nc.dram_tensor("netb", [2, 128], F32, kind="ExternalInput")
    lamb_d = nc.dram_tensor("lamb", [2, 128], F32, kind="ExternalInput")
    id_d = nc.dram_tensor("ident", [128, 128], BF16, kind="ExternalInput")
    out_d = nc.dram_tensor("out", [T, 256, 3136], F32, kind="ExternalOutput")

    with tile.TileContext(nc) as tc:
        with (
            tc.tile_pool(name="const", bufs=1) as cpool,
            tc.tile_pool(name="xp", bufs=1) as xpool,
            tc.tile_pool(name="big", bufs=1) as bigpool,
            tc.tile_pool(name="work", bufs=3) as wpool,
            tc.tile_pool(name="small", bufs=1) as spool,
            tc.tile_pool(name="cpsum", bufs=6, space="PSUM") as cpsum,
            tc.tile_pool(name="spsum", bufs=2, space="PSUM") as spsum,
        ):
            # ---- weights/constants; co_t=0 weight block first so PE can start
            id_sb = cpool.tile([128, 128], BF16)
            nc.sync.dma_start(out=id_sb[:], in_=id_d.ap())
            wt_sb = cpool.tile([128, 36, 128], BF16)
            nc.sync.dma_start(out=wt_sb[:, 0:18], in_=wt_d.ap()[:, 0:18])
            # all 8 frames resident; frame0 split row-wise so the first conv
            # tiles (rows 0-9) unblock as early as possible
            xin = [xpool.tile([128, 2, PADSZ], BF16, tag=f"xin{f}",
                              name=f"xin{f}") for f in range(T)]
            for r0, r1 in ((0, 16), (16, 32), (32, 44), (44, 58)):
                sl0 = slice(r0 * HP, r1 * HP)
                for ci in range(2):
                    nc.sync.dma_start(out=xin[0][:, ci, sl0],
                                      in_=x_d.ap()[0, ci][:, sl0])
            nc.sync.dma_start(out=wt_sb[:, 18:36], in_=wt_d.ap()[:, 18:36])
            for f in range(1, T):
                nc.sync.dma_start(
                    out=xin[f][:], in_=x_d.ap()[f].rearrange("t p m -> p t m"))
            wg_sb = cpool.tile([128, 48, 128], BF16)
            for h in range(2):
                nc.sync.dma_start(out=wg_sb[:, h * 24:(h + 1) * 24],
                                  in_=wg_d.ap()[:, h * 24:(h + 1) * 24])
            lamw_sb = cpool.tile([128, 4, 128], BF16)
            nc.sync.dma_start(out=lamw_sb[:], in_=lamw_d.ap().rearrange("w p m -> p w m"))
            w1t_sb = cpool.tile([8, 16], BF16)
            nc.sync.dma_start(out=w1t_sb[:], in_=w1t_d.ap())
            w2t_sb = cpool.tile([16, 3], BF16)
            nc.sync.dma_start(out=w2t_sb[:], in_=w2t_d.ap())
            bns_sb = cpool.tile([16, 1], F32)
            nc.sync.dma_start(out=bns_sb[:], in_=bns_d.ap())
            bnsh_sb = cpool.tile([16, 1], F32)
            nc.sync.dma_start(out=bnsh_sb[:], in_=bnsh_d.ap())
            netb_sb = cpool.tile([128, 2], F32)
            nc.sync.dma_start(out=netb_sb[:], in_=netb_d.ap().rearrange("t p -> p t"))
            lamb_sb = cpool.tile([128, 2], F32)
            nc.sync.dma_start(out=lamb_sb[:], in_=lamb_d.ap().rearrange("t p -> p t"))

            # warm the exp table set while ACT is idle (sigmoid is computed
            # via exp+reciprocal so only one table set is ever needed)
            warm = spool.tile([1, 2], F32)
            nc.vector.memset(warm[:], 0.0)
            nc.scalar.activation(out=warm[:, 0:1], in_=warm[:, 1:2], func=AF.Exp)
            # warm the PE HAM clock gate with dummy matmuls during the DMA
            # lead-in so the first conv tiles run at 2.4 GHz, not 1.2
            wps = spsum.tile([128, 64], F32, tag="sp", name="warm_ps")
            for i in range(16):
                nc.tensor.matmul(wps[:], id_sb[:], id_sb[:, 0:64],
                                 start=(i == 0), stop=(i == 15))

            # ---------------- prepass: strip sums -> T[ci, dy, dx, f] --------
            T_all = spool.tile([128, 2, 9, T], F32)   # taps dy*3+dx
            ST = spool.tile([128, 2, 4, 1], F32)      # r55, r0, c55, c0 (per-frame)
            S_sb = spool.tile([128, T, 2], F32)
            zeros9 = spool.tile([128, 3, 3, 1], F32)
            nc.vector.memset(zeros9[:], 0.0)

            def strip_reduce(dst, src):
                nc.vector.tensor_reduce(out=dst, in_=src,
                                        axis=mybir.AxisListType.X, op=OP.add)

            for f in range(T):
                strip_reduce(S_sb[:, f], xin[f][:])
                for ci in range(2):
                    xf = xin[f][:, ci]
                    strip_reduce(ST[:, ci, 0],
                                 xf[:, 56 * HP:57 * HP].rearrange("p (a c) -> p a c", a=1))
                    strip_reduce(ST[:, ci, 1],
                                 xf[:, 1 * HP:2 * HP].rearrange("p (a c) -> p a c", a=1))
                    xcols = xf.rearrange("p (r c) -> p c r", c=HP)
                    strip_reduce(ST[:, ci, 2], xcols[:, 56:57, :])
                    strip_reduce(ST[:, ci, 3], xcols[:, 1:2, :])
                    Tv = T_all[:, ci].rearrange("p (dy dx) f -> p dy dx f", dx=3)
                    nc.vector.tensor_scalar_add(
                        out=Tv[:, :, :, f:f + 1], in0=zeros9[:],
                        scalar1=S_sb[:, f, ci:ci + 1])
                    nc.vector.tensor_scalar_sub(
                        out=Tv[:, 0:1, :, f:f + 1], in0=Tv[:, 0:1, :, f:f + 1],
                        scalar1=ST[:, ci, 0])
                    nc.vector.tensor_scalar_sub(
                        out=Tv[:, 2:3, :, f:f + 1], in0=Tv[:, 2:3, :, f:f + 1],
                        scalar1=ST[:, ci, 1])
                    nc.vector.tensor_scalar_sub(
                        out=Tv[:, :, 0:1, f:f + 1], in0=Tv[:, :, 0:1, f:f + 1],
                        scalar1=ST[:, ci, 2])
                    nc.vector.tensor_scalar_sub(
                        out=Tv[:, :, 2:3, f:f + 1], in0=Tv[:, :, 2:3, f:f + 1],
                        scalar1=ST[:, ci, 3])
                    for (dy, dx, off) in ((0, 0, 56 * HP + 56), (0, 2, 56 * HP + 1),
                                          (2, 0, 1 * HP + 56), (2, 2, 1 * HP + 1)):
                        nc.vector.tensor_add(
                            out=Tv[:, dy, dx, f:f + 1], in0=Tv[:, dy, dx, f:f + 1],
                            in1=xf[:, off:off + 1])
            Tb = spool.tile([128, 2, 9, T], BF16)
            nc.vector.tensor_copy(out=Tb[:], in_=T_all[:])

            # ring of 4 frame outputs (phase-2 for frame p needs p-1, p, p+1)
            oraw = [bigpool.tile([128, 2, 3136], BF16, tag="oraw", name="oraw",
                                 bufs=4) for _ in range(T)]

            # ---------------- direct conv frame ----------------
            def conv_frame(f, slots=None, sp_orders=None):
                for co_t in range(2):
                    sp_seq = (sp_orders or {}).get(co_t, range(NSP))
                    for sp in sp_seq:
                        y0 = sp * 8
                        ct = cpsum.tile([128, 512], F32, tag="conv", name="ct")
                        idx = 0
                        for ci_t in range(2):
                            xv = xin[f][:, ci_t].rearrange("p (r c) -> p r c", c=HP)
                            for dy in range(3):
                                for dx in range(3):
                                    w = co_t * 18 + ci_t * 9 + dy * 3 + dx
                                    nc.tensor.matmul(
                                        ct[:, :TW],
                                        wt_sb[:, w],
                                        xv[:, y0 + dy:y0 + dy + 8, dx:dx + 56],
                                        start=(idx == 0), stop=(idx == 17))
                                    idx += 1
                        nc.scalar.activation(
                            out=oraw[f][:, co_t, sp * TW:(sp + 1) * TW],
                            in_=ct[:, :TW], func=AF.Copy)
                        if slots and (co_t, sp) in slots:
                            for fn in slots[(co_t, sp)]:
                                fn()

            # ---------------- Winograd F(2,3)-H frame (30 MM + 2 DVE) -------
            # row-pair chunks of 8,8,8,4 pairs -> matmul N of 448,448,448,224
            # (N=448 matches the direct conv, where per-MM overhead is ~2ns)
            PP = (8, 8, 8, 4)
            PB = (0, 8, 16, 24)

            def wino_v(f, c):
                # V planes (4 per ci) for row-pair chunk c, as 2x-mode TT ops
                pp, b = PP[c], PB[c]
                vt = [[wpool.tile([128, 8, HP], BF16, tag=f"V{ci}{a}",
                                  name=f"V{ci}{a}", bufs=2)
                       for a in range(4)] for ci in range(2)]
                for ci in range(2):
                    xv4 = xin[f][:, ci].rearrange("p (i par w) -> p i par w",
                                                  par=2, w=HP)
                    e0 = xv4[:, b:b + pp, 0, :]        # rows 2i   (d0)
                    o0 = xv4[:, b:b + pp, 1, :]        # rows 2i+1 (d1)
                    e1 = xv4[:, b + 1:b + pp + 1, 0, :]  # rows 2i+2 (d2)
                    o1 = xv4[:, b + 1:b + pp + 1, 1, :]  # rows 2i+3 (d3)
                    nc.vector.tensor_sub(out=vt[ci][0][:, :pp], in0=e0, in1=e1)
                    nc.vector.tensor_add(out=vt[ci][1][:, :pp], in0=o0, in1=e1)
                    nc.vector.tensor_sub(out=vt[ci][2][:, :pp], in0=e1, in1=o0)
                    nc.vector.tensor_sub(out=vt[ci][3][:, :pp], in0=o0, in1=o1)
                return vt

            def wino_block(f, c, co_t, vt):
                pp, b = PP[c], PB[c]
                NN = 56 * pp
                if True:
                    if True:
                        E = cpsum.tile([128, 512], F32, tag="conv", name="E")
                        idx = 0
                        for g, a in ((0, 0), (1, 1)):
                            for ci_t in range(2):
                                for dx in range(3):
                                    nc.tensor.matmul(
                                        E[:, :NN],
                                        wg_sb[:, co_t * 24 + g * 6 + ci_t * 3 + dx],
                                        vt[ci_t][a][:, :pp, dx:dx + 56],
                                        start=(idx == 0), stop=(idx == 11))
                                    idx += 1
                        M2 = cpsum.tile([128, 512], F32, tag="conv", name="M2")
                        idx = 0
                        for ci_t in range(2):
                            for dx in range(3):
                                nc.tensor.matmul(
                                    M2[:, :NN],
                                    wg_sb[:, co_t * 24 + 2 * 6 + ci_t * 3 + dx],
                                    vt[ci_t][2][:, :pp, dx:dx + 56],
                                    start=(idx == 0), stop=(idx == 5))
                                idx += 1
                        O = cpsum.tile([128, 512], F32, tag="conv", name="O")
                        idx = 0
                        for g, a in ((1, 1), (3, 3)):
                            for ci_t in range(2):
                                for dx in range(3):
                                    nc.tensor.matmul(
                                        O[:, :NN],
                                        wg_sb[:, co_t * 24 + g * 6 + ci_t * 3 + dx],
                                        vt[ci_t][a][:, :pp, dx:dx + 56],
                                        start=(idx == 0), stop=(idx == 11))
                                    idx += 1
                        # output transform: y_even = E + M2, y_odd = O - M2.
                        # Stage all three planes into SBUF bf16 via the idle
                        # ACT engine so the DVE combines run in 2x mode (and
                        # PSUM banks free earlier).
                        Ec = wpool.tile([128, 448], BF16, tag="M2c",
                                        name="Ec", bufs=4)
                        nc.scalar.activation(out=Ec[:, :NN], in_=E[:, :NN],
                                             func=AF.Copy)
                        M2c = wpool.tile([128, 448], BF16, tag="M2c",
                                         name="M2c", bufs=4)
                        nc.scalar.activation(out=M2c[:, :NN], in_=M2[:, :NN],
                                             func=AF.Copy)
                        Oc = wpool.tile([128, 448], BF16, tag="M2c",
                                        name="Oc", bufs=4)
                        nc.scalar.activation(out=Oc[:, :NN], in_=O[:, :NN],
                                             func=AF.Copy)
                        ov = oraw[f][:, co_t].rearrange(
                            "p (i par w) -> p i par w", par=2, w=56)
                        Ev = Ec[:, :NN].rearrange("p (i w) -> p i w", w=56)
                        Ov = Oc[:, :NN].rearrange("p (i w) -> p i w", w=56)
                        M2v = M2c[:, :NN].rearrange("p (i w) -> p i w", w=56)
                        nc.vector.tensor_add(
                            out=ov[:, b:b + pp, 0, :], in0=Ev, in1=M2v)
                        nc.vector.tensor_sub(
                            out=ov[:, b:b + pp, 1, :], in0=Ov, in1=M2v)

            def conv_frame_wino(f, ph2s=(), vt0=None, next_f=None):
                # ph2s: phase-2 chunk callables keyed by (c, co_t) slot
                vt = vt0 if vt0 is not None else wino_v(f, 0)
                nxt_ret = None
                for c in range(4):
                    vt_next = None
                    if c < 3:
                        vt_next = wino_v(f, c + 1)
                    elif next_f is not None:
                        nxt_ret = wino_v(next_f, 0)
                    for co_t in range(2):
                        wino_block(f, c, co_t, vt)
                        for (cc, cco), fn in ph2s:
                            if cc == c and cco == co_t:
                                fn()
                    if vt_next is not None:
                        vt = vt_next
                return nxt_ret

            # ---------------- small ops (gate chain), piecewise -------------
            SM = {}

            def p1_pooled_mms():
                pooled_ps = spsum.tile([128, 2, T], F32, tag="sp", name="pooled_ps")
                for co_t in range(2):
                    idx = 0
                    for ci_t in range(2):
                        for tap in range(9):
                            w = co_t * 18 + ci_t * 9 + tap
                            nc.tensor.matmul(
                                pooled_ps[:, co_t], wt_sb[:, w],
                                Tb[:, ci_t, tap, :],
                                start=(idx == 0), stop=(idx == 17))
                            idx += 1
                SM['pooled_ps'] = pooled_ps

            def p2():
                pooled_sum = spool.tile([128, 2, T], F32)
                nc.vector.tensor_copy(out=pooled_sum[:], in_=SM['pooled_ps'])
                total = spool.tile([128, 2], F32)
                nc.vector.tensor_reduce(
                    out=total[:], in_=pooled_sum[:], axis=mybir.AxisListType.X,
                    op=OP.add)
                xgpre = spool.tile([128, 2], BF16)
                for t in range(2):
                    nc.vector.tensor_scalar(
                        out=xgpre[:, t:t + 1], in0=total[:, t:t + 1],
                        scalar1=1.0 / (T * 3136.0), scalar2=netb_sb[:, t:t + 1],
                        op0=OP.mult, op1=OP.add)
                xg_ps = spsum.tile([128, 2], F32, tag="sp", name="xg_ps")
                for ct_ in range(2):
                    for kt in range(2):
                        nc.tensor.matmul(
                            xg_ps[:, ct_:ct_ + 1], lamw_sb[:, kt * 2 + ct_],
                            xgpre[:, kt:kt + 1], start=(kt == 0), stop=(kt == 1))
                SM['pooled_sum'] = pooled_sum
                SM['xg_ps'] = xg_ps

            def p3():
                xg = spool.tile([128, 2], F32)
                for t in range(2):
                    nc.scalar.activation(
                        out=xg[:, t:t + 1], in_=SM['xg_ps'][:, t:t + 1],
                        func=AF.Identity, bias=lamb_sb[:, t:t + 1])
                bxg = spool.tile([128, 2], F32)
                nc.vector.tensor_add(out=bxg[:], in0=netb_sb[:], in1=xg[:])
                pooled = spool.tile([128, 2, T], F32)
                for t in range(2):
                    nc.vector.tensor_scalar(
                        out=pooled[:, t], in0=SM['pooled_sum'][:, t],
                        scalar1=1.0 / 3136.0, scalar2=bxg[:, t:t + 1],
                        op0=OP.mult, op1=OP.add)
                pooled_bf = spool.tile([128, 2, T], BF16)
                nc.vector.tensor_copy(out=pooled_bf[:], in_=pooled[:])
                pT_ps = spsum.tile([8, 256], BF16, tag="sp", name="pT_ps")
                for t in range(2):
                    nc.tensor.transpose(
                        pT_ps[:, t * 128:(t + 1) * 128], pooled_bf[:, t], id_sb[:])
                SM.update(bxg=bxg, pooled=pooled, pT_ps=pT_ps)

            def p4():
                pooledT = spool.tile([8, 256], BF16)
                nc.vector.tensor_copy(out=pooledT[:], in_=SM['pT_ps'])
                hdn_ps = spsum.tile([16, 256], F32, tag="sp", name="hdn_ps")
                nc.tensor.matmul(hdn_ps[:], w1t_sb[:], pooledT[:], start=True,
                                 stop=True)
                hdnr = spool.tile([16, 256], BF16)
                nc.scalar.activation(
                    out=hdnr[:], in_=hdn_ps[:], func=AF.Relu,
                    scale=bns_sb[:, 0:1], bias=bnsh_sb[:, 0:1])
                SM['hdnr'] = hdnr

            def p5():
                lgT_ps = spsum.tile([3, 256], F32, tag="sp", name="lgT_ps")
                nc.tensor.matmul(lgT_ps[:], w2t_sb[:], SM['hdnr'], start=True,
                                 stop=True)
                lgT = spool.tile([3, 256], BF16)
                nc.vector.tensor_copy(out=lgT[:], in_=lgT_ps[:])
                SM['lgT'] = lgT

            def p6():
                ew = spool.tile([128, 2, 3], F32)
                for t in range(2):
                    lg_ps = spsum.tile([128, 3], BF16, tag="sp", name="lg_ps")
                    nc.tensor.transpose(
                        lg_ps[:], SM['lgT'][:, t * 128:(t + 1) * 128],
                        id_sb[0:3, 0:3])
                    nc.scalar.activation(out=ew[:, t], in_=lg_ps[:], func=AF.Exp)
                SM['ew'] = ew

            def p7():
                ew = SM['ew']
                es = spool.tile([128, 2], F32)
                nc.vector.tensor_reduce(
                    out=es[:], in_=ew[:], axis=mybir.AxisListType.X, op=OP.add)
                esr = spool.tile([128, 2], F32)
                nc.vector.reciprocal(out=esr[:], in_=es[:])
                wgt = spool.tile([128, 2, 3], F32)
                for t in range(2):
                    nc.vector.tensor_scalar_mul(
                        out=wgt[:, t], in0=ew[:, t], scalar1=esr[:, t:t + 1])
                m = spool.tile([128, 2, T], F32)
                pooled = SM['pooled']
                for t in range(2):
                    nc.vector.tensor_scalar_mul(
                        out=m[:, t], in0=pooled[:, t], scalar1=wgt[:, t, 1:2])
                    nc.vector.scalar_tensor_tensor(
                        out=m[:, t, 1:T], in0=pooled[:, t, 0:T - 1],
                        scalar=wgt[:, t, 0:1], in1=m[:, t, 1:T],
                        op0=OP.mult, op1=OP.add)
                    nc.vector.scalar_tensor_tensor(
                        out=m[:, t, 0:T - 1], in0=pooled[:, t, 1:T],
                        scalar=wgt[:, t, 2:3], in1=m[:, t, 0:T - 1],
                        op0=OP.mult, op1=OP.add)
                y = spool.tile([128, 2, T], F32)
                nc.vector.memset(y[:], 0.0)
                for t in range(2):
                    nc.vector.tensor_sub(
                        out=y[:, t, 0:T - 1], in0=m[:, t, 1:T], in1=m[:, t, 0:T - 1])
                y_bf = spool.tile([128, 2, T], BF16)
                nc.vector.tensor_copy(out=y_bf[:], in_=y[:])
                yT_ps = spsum.tile([8, 256], BF16, tag="sp", name="yT_ps")
                for t in range(2):
                    nc.tensor.transpose(
                        yT_ps[:, t * 128:(t + 1) * 128], y_bf[:, t], id_sb[:])
                SM.update(wgt=wgt, yT_ps=yT_ps)

            def p8():
                yT = spool.tile([8, 256], F32)
                nc.vector.tensor_copy(out=yT[:], in_=SM['yT_ps'])
                ycT = spool.tile([8, 256], F32)
                nc.vector.tensor_scalar_mul(out=ycT[:], in0=yT[:], scalar1=float(me[1]))
                nc.vector.scalar_tensor_tensor(
                    out=ycT[:, 1:256], in0=yT[:, 0:255], scalar=float(me[0]),
                    in1=ycT[:, 1:256], op0=OP.mult, op1=OP.add)
                nc.vector.scalar_tensor_tensor(
                    out=ycT[:, 0:255], in0=yT[:, 1:256], scalar=float(me[2]),
                    in1=ycT[:, 0:255], op0=OP.mult, op1=OP.add)
                # sigmoid via exp + reciprocal (stays in the exp table set);
                # reuse ycT/yT buffers to save SBUF
                nc.scalar.activation(out=ycT[:], in_=ycT[:], func=AF.Exp,
                                     scale=-1.0)
                nc.vector.tensor_scalar_add(out=ycT[:], in0=ycT[:], scalar1=1.0)
                nc.vector.reciprocal(out=yT[:], in_=ycT[:])
                gateT = spool.tile([8, 256], BF16)
                nc.vector.tensor_copy(out=gateT[:], in_=yT[:])
                SM['gateT'] = gateT

            def p9():
                wgt, bxg = SM['wgt'], SM['bxg']
                gate_c = spool.tile([128, 2, T], F32)
                for t in range(2):
                    g_ps = spsum.tile([128, 8], BF16, tag="sp", name="g_ps")
                    nc.tensor.transpose(
                        g_ps[:], SM['gateT'][:, t * 128:(t + 1) * 128],
                        id_sb[0:8, 0:8])
                    nc.vector.tensor_copy(out=gate_c[:, t], in_=g_ps[:])
                g0 = spool.tile([128, 2, T], F32)
                g1 = spool.tile([128, 2, T], F32)
                g2 = spool.tile([128, 2, T], F32)
                g = [g0, g1, g2]
                for k in range(3):
                    for t in range(2):
                        nc.vector.tensor_scalar_mul(
                            out=g[k][:, t], in0=gate_c[:, t], scalar1=wgt[:, t, k:k + 1])
                goffs = spool.tile([128, 2, T], F32)
                w01 = spool.tile([128, 2], F32)
                w12 = spool.tile([128, 2], F32)
                for t in range(2):
                    nc.vector.tensor_scalar_mul(
                        out=goffs[:, t], in0=gate_c[:, t], scalar1=bxg[:, t:t + 1])
                    nc.vector.tensor_add(
                        out=w12[:, t:t + 1], in0=wgt[:, t, 1:2], in1=wgt[:, t, 2:3])
                    nc.vector.tensor_add(
                        out=w01[:, t:t + 1], in0=wgt[:, t, 0:1], in1=wgt[:, t, 1:2])
                    nc.vector.tensor_mul(
                        out=goffs[:, t, 0:1], in0=goffs[:, t, 0:1], in1=w12[:, t:t + 1])
                    nc.vector.tensor_mul(
                        out=goffs[:, t, 7:8], in0=goffs[:, t, 7:8], in1=w01[:, t:t + 1])
                SM.update(g0=g0, g1=g1, g2=g2, goffs=goffs)

            # ---------------- phase 2 ----------------
            # 448-wide chunks; bf16 intermediates keep the DVE in 2x/4x
            # modes, the middle tap runs on the mostly-idle ACT engine.
            CH = [(i * 448, (i + 1) * 448) for i in range(7)]
            DMAC = [0]  # round-robin output-DMA engine selector

            def phase2_chunk(p, co_t, chi):
                g0, g1, g2, goffs = SM['g0'], SM['g1'], SM['g2'], SM['goffs']
                g = [g0, g1, g2]
                c0, c1 = CH[chi]
                sl = slice(c0, c1)

                def o(ff):
                    return oraw[ff][:, co_t, sl]
                fin = wpool.tile([128, 448], F32, tag="fin", name="fin", bufs=2)
                A = wpool.tile([128, 448], BF16, tag="A", name="A", bufs=2)
                if p == 0 or p == T - 1:
                    ve = nc.vector
                    fa, ka, fb, kb = (0, 1, 1, 2) if p == 0 else (T - 2, 0, T - 1, 1)
                    ve.tensor_scalar(
                        out=A[:], in0=o(fa),
                        scalar1=g[ka][:, co_t, p:p + 1],
                        scalar2=goffs[:, co_t, p:p + 1],
                        op0=OP.mult, op1=OP.add)
                    ve.scalar_tensor_tensor(
                        out=fin[:], in0=o(fb),
                        scalar=g[kb][:, co_t, p:p + 1],
                        in1=A[:], op0=OP.mult, op1=OP.add)
                else:
                    nc.vector.tensor_scalar(
                        out=A[:], in0=o(p - 1),
                        scalar1=g0[:, co_t, p:p + 1],
                        scalar2=goffs[:, co_t, p:p + 1],
                        op0=OP.mult, op1=OP.add)
                    Bp = wpool.tile([128, 448], BF16, tag="Bp", name="Bp",
                                    bufs=2)
                    nc.scalar.mul(Bp[:], o(p), g1[:, co_t, p:p + 1])
                    nc.vector.tensor_add(out=A[:], in0=A[:], in1=Bp[:])
                    nc.vector.scalar_tensor_tensor(
                        out=fin[:], in0=o(p + 1),
                        scalar=g2[:, co_t, p:p + 1],
                        in1=A[:], op0=OP.mult, op1=OP.add)
                eng = nc.sync if (DMAC[0] % 2 == 0) else nc.gpsimd
                DMAC[0] += 1
                eng.dma_start(
                    out=out_d.ap()[p, co_t * 128:(co_t + 1) * 128, sl],
                    in_=fin[:])

            def ph2(p, co_t, chi):
                return lambda: phase2_chunk(p, co_t, chi)

            # split phase-2: the first two taps (pre) run one conv frame
            # earlier than the last tap (fin), with the bf16 partial parked
            # in the long-dead xin[0]/xin[1] buffers (tile-tag reuse).
            PRE = {}

            def pre_alloc(p):
                # xin[0..2] buffers are dead by the time any pre is written;
                # p%3 cycling keeps every reuse chain cross-frame (no
                # same-window WAR between a pre write and a fin read)
                PRE[p] = xpool.tile([128, 2, PADSZ], BF16,
                                    tag=f"xin{p % 3}", name=f"pre{p}")

            def ph2_pre(p, co_t, chi):
                def emit():
                    g0, g1, goffs = SM['g0'], SM['g1'], SM['goffs']
                    sl = slice(chi * 448, (chi + 1) * 448)
                    buf = PRE[p][:, co_t, sl]

                    def o(ff):
                        return oraw[ff][:, co_t, sl]
                    if p == T - 1:
                        nc.vector.tensor_scalar(
                            out=buf, in0=o(T - 2),
                            scalar1=g0[:, co_t, p:p + 1],
                            scalar2=goffs[:, co_t, p:p + 1],
                            op0=OP.mult, op1=OP.add)
                    else:
                        nc.vector.tensor_scalar(
                            out=buf, in0=o(p - 1),
                            scalar1=g0[:, co_t, p:p + 1],
                            scalar2=goffs[:, co_t, p:p + 1],
                            op0=OP.mult, op1=OP.add)
                        Bp = wpool.tile([128, 448], BF16, tag="Bp", name="Bp",
                                        bufs=2)
                        nc.scalar.mul(Bp[:], o(p), g1[:, co_t, p:p + 1])
                        nc.vector.tensor_add(out=buf, in0=buf, in1=Bp[:])
                return emit

            def ph2_fin(p, co_t, chi):
                def emit():
                    g1, g2 = SM['g1'], SM['g2']
                    sl = slice(chi * 448, (chi + 1) * 448)
                    buf = PRE[p][:, co_t, sl]
                    ks = g1 if p == T - 1 else g2
                    fin = wpool.tile([128, 448], F32, tag="fin", name="fin",
                                     bufs=2)
                    nc.vector.scalar_tensor_tensor(
                        out=fin[:], in0=oraw[p + 1 if p < T - 1 else p][:, co_t, sl],
                        scalar=ks[:, co_t, p:p + 1],
                        in1=buf, op0=OP.mult, op1=OP.add)
                    eng = nc.sync if (DMAC[0] % 2 == 0) else nc.gpsimd
                    DMAC[0] += 1
                    eng.dma_start(
                        out=out_d.ap()[p, co_t * 128:(co_t + 1) * 128, sl],
                        in_=fin[:])
                return emit

            # ---------------- schedule ----------------
            conv_frame(0)
            conv_frame(1)
            p1_pooled_mms()
            # frame 2: Winograd; the gate chain rides its 8 (chunk, co) slots
            sm = [((0, 0), p2), ((0, 1), p3), ((1, 0), p4), ((1, 1), p5),
                  ((2, 0), p6), ((2, 1), p7), ((3, 0), p8), ((3, 1), p9)]
            conv_frame_wino(2, ph2s=sm)
            # frame 3: direct; carries phase-2 of frames 0 and 1 (chunk chi
            # aligns exactly with sp tile chi)
            sl3 = {}
            for co in range(2):
                for sp in range(NSP):
                    sl3[(co, sp)] = [ph2(0, co, sp), ph2(1, co, sp)]
                    if sp <= 4:  # f3 has DVE slack: take most of ph2(2) too
                        sl3[(co, sp)].append(ph2(2, co, sp))
            conv_frame(3, slots=sl3)
            # frames 4-7: Winograd; phase-2 chunk chi maps to row-chunk
            # c = cmap[chi].  Spread the ph2(2)/(3) backlog over f4/f5.
            cmap = (0, 0, 1, 1, 2, 2, 3)
            plan = {4: [(2, (5, 6)), (3, range(7))],
                    5: [(4, range(7))]}
            pre_alloc(5)
            vt_pre = None
            for f in (4, 5):
                ph2s = []
                for p, chis in plan[f]:
                    for chi in chis:
                        for co in range(2):
                            ph2s.append(((cmap[chi], co), ph2(p, co, chi)))
                if f == 5:
                    # taps 0/1 of ph2(5) run here; only the last tap stays
                    # in the (saturated) merged f6/f7 region
                    for chi in range(7):
                        for co in range(2):
                            ph2s.append(((cmap[chi], co), ph2_pre(5, co, chi)))
                vt_pre = conv_frame_wino(f, ph2s=ph2s, vt0=vt_pre,
                                         next_f=5 if f == 4 else 6)
            # frames 6 and 7 chunk-interleaved: phase-2 of frames 5/6/7
            # spreads over the combined ~80us window instead of piling the
            # last two gating frames into frame 7's window alone.
            chis_at = {c: [chi for chi in range(7) if cmap[chi] == c]
                       for c in range(4)}
            vt6, vt7 = vt_pre, wino_v(7, 0)
            for c in range(4):
                n6 = wino_v(6, c + 1) if c < 3 else None
                n7 = wino_v(7, c + 1) if c < 3 else None
                for co in range(2):
                    wino_block(6, c, co, vt6)
                    for chi in chis_at[c]:
                        ph2_fin(5, co, chi)()
                for co in range(2):
                    wino_block(7, c, co, vt7)
                    for chi in chis_at[c]:
                        phase2_chunk(6, co, chi)
                        phase2_chunk(7, co, chi)
                if n6 is not None:
                    vt6, vt7 = n6, n7

    nc.compile()
    return nc


def _prep(inputs):
    x = np.asarray(inputs["x"], np.float32)          # (64,256,56,56)
    net_w = np.asarray(inputs["net_w"], np.float32)  # (256,256,3,3)
    net_b = np.asarray(inputs["net_b"], np.float32)
    lam_w = np.asarray(inputs["lam_w"], np.float32)
    lam_b = np.asarray(inputs["lam_b"], np.float32)
    mlp_w1 = np.asarray(inputs["mlp_w1"], np.float32)  # (16,8)
    mlp_w2 = np.asarray(inputs["mlp_w2"], np.float32)  # (3,16)
    bn_g = np.asarray(inputs["bn_gamma"], np.float32)
    bn_b = np.asarray(inputs["bn_beta"], np.float32)
    bn_m = np.asarray(inputs["bn_mean"], np.float32)
    bn_v = np.asarray(inputs["bn_var"], np.float32)
    me_w = np.asarray(inputs["me_w"], np.float32)

    bf = ml_dtypes.bfloat16
    xs = x.reshape(NCORES, T, 2, 128, 56, 56)
    xpad = np.zeros((NCORES, T, 2, 128, HP, HP), dtype=bf)
    xpad[:, :, :, :, 1:57, 1:57] = xs.astype(bf)
    xpad = np.ascontiguousarray(xpad.reshape(NCORES, T, 2, 128, PADSZ))

    # wt[p=ci, w_idx, m=co] with w_idx = co_t*18 + ci_t*9 + dy*3 + dx
    wtb = net_w.reshape(2, 128, 2, 128, 3, 3)          # co_t co ci_t ci dy dx
    wt = wtb.transpose(3, 0, 2, 4, 5, 1)               # ci co_t ci_t dy dx co
    wt = np.ascontiguousarray(wt.reshape(128, 36, 128).astype(bf))
    # Winograd F(2,3)-H weights: g0=w[dy0], g1=(w0+w1+w2)/2, g2=(w0-w1+w2)/2,
    # g3 = -w[dy2] (sign folded for the O = M1 - M3 accumulation).
    # wg[p=ci, slot, m=co], slot = co_t*24 + g*6 + ci_t*3 + dx
    w0, w1, w2 = wtb[..., 0, :], wtb[..., 1, :], wtb[..., 2, :]
    gstk = np.stack([w0, 0.5 * (w0 + w1 + w2), 0.5 * (w0 - w1 + w2), -w2],
                    axis=2)                             # co_t co g ci_t ci dx
    wg = gstk.transpose(4, 0, 2, 3, 5, 1)               # ci co_t g ci_t dx co
    wg = np.ascontiguousarray(wg.reshape(128, 48, 128).astype(bf))
    lamw = lam_w.T.reshape(2, 128, 2, 128).transpose(0, 2, 1, 3)
    lamw = np.ascontiguousarray(lamw.reshape(4, 128, 128).astype(bf))
    w1t = np.ascontiguousarray(mlp_w1.T.astype(bf))      # (8,16)
    w2t = np.ascontiguousarray(mlp_w2.T.astype(bf))      # (16,3)
    bns = (bn_g / np.sqrt(bn_v + 1e-5)).astype(np.float32).reshape(16, 1)
    bnsh = (bn_b - bn_m * bns[:, 0]).astype(np.float32).reshape(16, 1)
    netb = np.ascontiguousarray(net_b.reshape(2, 128))
    lamb = np.ascontiguousarray(lam_b.reshape(2, 128))
    ident = np.eye(128, dtype=bf)

    common = dict(wt=wt, wg=wg, lamw=lamw, w1t=w1t, w2t=w2t, bns=bns,
                  bnsh=bnsh, netb=netb, lamb=lamb, ident=ident)
    in_maps = [dict(x=xpad[i], **common) for i in range(NCORES)]
    return in_maps, tuple(float(v) for v in me_w)


def kernel(**inputs):
    in_maps, me = _prep(inputs)
    nc = _CACHE.get(me)
    if nc is None:
        nc = _build(me)
        _CACHE[me] = nc
    res = run_bass_kernel_spmd(nc, in_maps, core_ids=list(range(NCORES)))
    out = np.stack([res.results[i]["out"] for i in range(NCORES)])  # (8,8,256,3136)
    return np.ascontiguousarray(out.reshape(64, 256, 56, 56))
